# revision 3
# baseline (speedup 1.0000x reference)
"""MemoryBank MoE-routing kernel for 8 Trainium2 NeuronCores — v5.

Reference semantics (B=16, S=2048, D=1024, M=512, T=256, K=8):
    x0 = x[:, 0, :]                          # [B, D]
    scores = x0 @ memory_router              # [B, M]
    top_vals, top_idx = top_k(scores, 8)     # [B, K]
    w = softmax(top_vals)                    # [B, K]
    combined = sum_k w[b,k] * memory_tokens[top_idx[b,k]]   # [B, T, D]
    out = x;  out[:, 1:T+1, :] = combined

v5 design (v2/v4 post-mortem: both were bound by the DVE FMA chain — 16
serial fp32 scalar_tensor_tensor at 2.35 us each; a bf16 accumulator alone
does not engage the DVE 16-bit fast path, all three streams must be 16-bit):
  * Gathers are SWDGE indirect DMAs that CAST f32 -> bf16 inline, so the
    FMA chain runs all-bf16 (~2x DVE).
  * Routing runs in bf16: 8 two-column matmuls accumulate scores [2, M] in
    PSUM, two selector matmuls broadcast them to all 128 partitions, then
    the usual max/max_index/softmax.  Router chunks load f32 over both
    HWDGE rings and are cast to bf16 on the (idle) DVE.
  * Combined accumulates in bf16 and is written back as bf16 (half the
    bytes); the host upcasts to f32 during the unshard step
    (out = x.copy(); out[:,1:T+1] = combined).
"""

import numpy as np

import concourse.bass as bass
import concourse.bacc as bacc
import concourse.mybir as mybir
from concourse import tile
from concourse.bass_utils import run_bass_kernel_spmd

N_CORES = 8
B, S, D = 16, 2048, 1024
M, T = 512, 256
K = 8
B_LOC = B // N_CORES  # batches per core
KT = D // 128         # contraction tiles for the router matmul

F32 = mybir.dt.float32
BF16 = mybir.dt.bfloat16
U32 = mybir.dt.uint32

RUN_KWARGS = {}
LAST_RESULT = None


def build_program():
    nc = bacc.Bacc(
        "TRN2",
        target_bir_lowering=False,
        debug=False,
        enable_asserts=True,
        num_devices=N_CORES,
    )

    x0 = nc.dram_tensor("x0", [B_LOC, D], F32, kind="ExternalInput")
    mem = nc.dram_tensor("mem", [M, T, D], F32, kind="ExternalInput")
    router = nc.dram_tensor("router", [D, M], F32, kind="ExternalInput")
    cmb_out = nc.dram_tensor("cmb", [B_LOC, T, D], BF16, kind="ExternalOutput")

    with tile.TileContext(nc) as tc:
        with (
            tc.tile_pool(name="sbuf", bufs=1) as sp,
            tc.tile_pool(name="g0pool", bufs=6) as gp0,
            tc.tile_pool(name="g1pool", bufs=6) as gp1,
            tc.tile_pool(name="psum", bufs=1, space="PSUM") as pp,
        ):
            # ---- x0 over the sync HWDGE ring + DVE cast: keeps the serial
            # Q7 emission queue free so router chunk 0's SWDGE cast-load
            # starts immediately (x0 ahead of it cost ~2.2 us of emission).
            x0f = sp.tile([128, B_LOC * KT], F32)
            for b in range(B_LOC):
                nc.sync.dma_start(
                    out=x0f[:, b * KT : (b + 1) * KT],
                    in_=x0[b, :].rearrange("(kt p) -> p kt", p=128),
                )
            x0w = sp.tile([128, B_LOC * KT], BF16)
            nc.vector.tensor_copy(out=x0w[:], in_=x0f[:])
            x0w_v = x0w[:].rearrange("p (b kt) -> p b kt", b=B_LOC)

            rings = [nc.sync, nc.scalar]
            wts = [sp.tile([128, M], BF16, name=f"wt{kt}") for kt in range(KT)]
            # chunks 0-3: SWDGE cast-load straight to bf16 (Q7 queue, after x0);
            # chunks 4-7: f32 over both HWDGE rings, DVE-cast.  The two paths
            # run in parallel, halving the router load time.
            for kt in range(4):
                nc.gpsimd.dma_start(
                    out=wts[kt][:], in_=router[kt * 128 : (kt + 1) * 128, :]
                )
            wtfs = [sp.tile([128, M], F32, name=f"wtf{kt}") for kt in range(4, KT)]
            for i, kt in enumerate(range(4, KT)):
                rings[i % 2].dma_start(
                    out=wtfs[i][:], in_=router[kt * 128 : (kt + 1) * 128, :]
                )
                nc.vector.tensor_copy(out=wts[kt][:], in_=wtfs[i][:])

            # selector rows for the partition-broadcast matmuls:
            # sel[p, b*128 + i] = (p == b), as a NEFF-embedded const
            import ml_dtypes

            sel_np = np.zeros((B_LOC, B_LOC * 128), dtype=ml_dtypes.bfloat16)
            for b in range(B_LOC):
                sel_np[b, b * 128 : (b + 1) * 128] = 1.0
            sel_dram = nc.inline_tensor(sel_np, name="sel_const")
            sel = sp.tile([B_LOC, B_LOC * 128], BF16)
            nc.sync.dma_start(out=sel[:], in_=sel_dram[:, :])

            # iota for the per-partition gather row offsets
            iota = sp.tile([128, 1], mybir.dt.int32)
            nc.gpsimd.iota(iota[:], pattern=[[0, 1]], base=0, channel_multiplier=1)
            iotaf = sp.tile([128, 1], F32)
            nc.vector.tensor_copy(out=iotaf[:], in_=iota[:])

            # ---- scores [2, M]: 8 accumulating bf16 matmuls ----
            scores2_p = pp.tile([B_LOC, M], F32, name="scores2", tag="scores2")
            for kt in range(KT):
                nc.tensor.matmul(
                    out=scores2_p[:],
                    lhsT=x0w_v[:, :, kt],
                    rhs=wts[kt][:],
                    start=(kt == 0),
                    stop=(kt == KT - 1),
                )
            scores2 = sp.tile([B_LOC, M], BF16, name="scores2_sb")
            nc.vector.tensor_copy(out=scores2[:], in_=scores2_p[:])

            # ---- per batch: broadcast scores to 128 partitions, topk+softmax
            w_all = []
            ridu_all = []
            for b in range(B_LOC):
                scores_p = pp.tile([128, M], F32, name=f"scores{b}", tag=f"scores{b}")
                nc.tensor.matmul(
                    out=scores_p[:],
                    lhsT=sel[:, b * 128 : (b + 1) * 128],
                    rhs=scores2[:],
                    start=True,
                    stop=True,
                )
                vals = sp.tile([128, K], F32, name=f"vals{b}", tag=f"vals{b}")
                nc.vector.max(out=vals[:], in_=scores_p[:])
                idx = sp.tile([128, K], U32, name=f"idx{b}", tag=f"idx{b}")
                nc.vector.max_index(out=idx[:], in_max=vals[:], in_values=scores_p[:])

                negmax = sp.tile([128, 1], F32, name=f"negmax{b}", tag=f"negmax{b}")
                nc.vector.tensor_scalar_mul(negmax[:], vals[:, 0:1], -1.0)
                ex = sp.tile([128, K], F32, name=f"ex{b}", tag=f"ex{b}")
                ssum = sp.tile([128, 1], F32, name=f"ssum{b}", tag=f"ssum{b}")
                nc.scalar.activation(
                    out=ex[:],
                    in_=vals[:],
                    func=mybir.ActivationFunctionType.Exp,
                    bias=negmax[:, 0:1],
                    scale=1.0,
                    accum_out=ssum[:, 0:1],
                )
                rec = sp.tile([128, 1], F32, name=f"rec{b}", tag=f"rec{b}")
                nc.vector.reciprocal(rec[:], ssum[:])
                w = sp.tile([128, K], F32, name=f"w{b}", tag=f"w{b}")
                nc.vector.tensor_scalar(
                    out=w[:],
                    in0=ex[:],
                    scalar1=rec[:, 0:1],
                    scalar2=None,
                    op0=mybir.AluOpType.mult,
                )
                w_all.append(w)

                # rid[p, k] = idx[b,k]*(T/2) + p  (two t-rows per gather row)
                idxf = sp.tile([128, K], F32, name=f"idxf{b}", tag=f"idxf{b}")
                nc.vector.tensor_copy(out=idxf[:], in_=idx[:])
                ridf = sp.tile([128, K], F32, name=f"ridf{b}", tag=f"ridf{b}")
                nc.vector.scalar_tensor_tensor(
                    out=ridf[:],
                    in0=idxf[:],
                    scalar=float(T // 2),
                    in1=iotaf[:, 0:1].to_broadcast([128, K]),
                    op0=mybir.AluOpType.mult,
                    op1=mybir.AluOpType.add,
                )
                ridu = sp.tile([128, K], U32, name=f"ridu{b}", tag=f"ridu{b}")
                nc.vector.tensor_copy(out=ridu[:], in_=ridf[:])
                ridu_all.append(ridu)

            # ---- gather (SWDGE indirect, f32 -> bf16 cast) + bf16 FMA ----
            mem2 = mem[:, :, :].rearrange("m (t2 j) d -> (m t2) (j d)", j=2)
            cmbs = [
                sp.tile([128, 2 * D], BF16, name=f"cmb{b}", tag=f"cmb{b}")
                for b in range(B_LOC)
            ]
            gps = [gp0, gp1]
            for k in range(K):
                for b in range(B_LOC):
                    g = gps[b].tile([128, 2 * D], BF16, tag=f"g{b}")
                    nc.gpsimd.indirect_dma_start(
                        out=g[:],
                        out_offset=None,
                        in_=mem2,
                        in_offset=bass.IndirectOffsetOnAxis(
                            ap=ridu_all[b][:, k : k + 1], axis=0
                        ),
                    )
                    if k == 0:
                        nc.vector.tensor_scalar_mul(
                            cmbs[b][:], g[:], w_all[b][:, 0:1]
                        )
                    else:
                        nc.vector.scalar_tensor_tensor(
                            out=cmbs[b][:],
                            in0=g[:],
                            scalar=w_all[b][:, k : k + 1],
                            in1=cmbs[b][:],
                            op0=mybir.AluOpType.mult,
                            op1=mybir.AluOpType.add,
                        )

            # ---- write combined back (bf16); t-rows t = 2*p+j.
            for b in range(B_LOC):
                rings[b].dma_start(
                    out=cmb_out[b, :, :].rearrange("(p j) d -> p j d", j=2),
                    in_=cmbs[b][:].rearrange("p (j d) -> p j d", j=2),
                )

    nc.compile()
    return nc


def kernel(x, memory_tokens, memory_router):
    global LAST_RESULT
    x = np.asarray(x)
    mem = np.asarray(memory_tokens)
    router = np.asarray(memory_router)

    nc = build_program()
    in_maps = [
        {
            "x0": np.ascontiguousarray(x[c * B_LOC : (c + 1) * B_LOC, 0, :]),
            "mem": mem,
            "router": router,
        }
        for c in range(N_CORES)
    ]
    res = run_bass_kernel_spmd(nc, in_maps, list(range(N_CORES)), **RUN_KWARGS)
    LAST_RESULT = res

    combined = np.concatenate(
        [np.asarray(res.results[c]["cmb"]).astype(np.float32) for c in range(N_CORES)],
        axis=0,
    )  # [B, T, D]
    out = x.copy()
    out[:, 1 : T + 1, :] = combined
    return out
